# revision 5
# baseline (speedup 1.0000x reference)
"""GAT message-passing kernel for 8 Trainium2 NeuronCores (Bass/Tile).

Strategy ("route edges by dst ownership", gather-free):
  - Host bin-packs nodes into 160 blocks (<=128 nodes, <=2048 in-edges each);
    each core owns 20 blocks, so segment-softmax and scatter-sum are fully
    core-local (no collectives, no device gather).
  - The host ships per-edge operand streams in tile order: eft[e].T,
    nft[src[e]].T and nft[dst[e]].T (pure index-gathers of the inputs, all
    f16).  Every FLOP of the model runs on device:
      per 128-edge tile, one PSUM tile [e,136] accumulates three matmuls:
        etT @ [W2 | W2A2]          (edge features -> eW2 and logit part)
        nftsT @ [W1 | W1A2+Wa1]    (y1[src] and qa[src])
        nftdT @ [W3A2]             (r[dst], logit columns only)
    giving part = y1+eW2 (cols 0:128) and z = qa+eW2A2+r (cols 128:136).
  - Softmax without max-subtraction: u = exp(leaky(z) - 7) (shift-invariant,
    fp16-safe); messages msgu = part*u with u riding in cols 128:136; a
    per-tile one-hot P (built on-device by is_equal against an iota) scatters
    [msg | u] into the block accumulator psb = [sum u*part | sum u].
  - Per block: mn = agg/s, + y3 (nft_own@W3, computed on device, masked to
    nodes with in-edges), + nft residual, relu, store [node, feat].
"""

import sys
import numpy as np

for _p in ("/opt/trn_rl_repo",):
    if _p not in sys.path:
        sys.path.append(_p)

import concourse.bacc as bacc
import concourse.bass as bass
import concourse.mybir as mybir
from concourse.tile import TileContext
from concourse import bass_utils

F = 128
H = 8
DH = 16
F2 = F + H           # 136
NCORES = 8
NODE_CAP = 128       # node slots per block
BPC = 20             # blocks per core
NPC = BPC * NODE_CAP # 2560 node slots per core
NBLK = NCORES * BPC  # 160 blocks
EXP_SHIFT = 7.0      # u = exp(a - shift); softmax-invariant, keeps u in f16
DUMMY_SLOT = 200.0   # dstloc for dummy edges: is_equal never matches


def build_nc(tpb, has_bias):
    """tpb: tiles (of 128 edges) per block; edge cap per block = 128*tpb."""
    ntiles = BPC * tpb
    epc = ntiles * 128
    dt = mybir.dt
    AOP = mybir.AluOpType

    nc = bacc.Bacc("TRN2", target_bir_lowering=False, debug=False,
                   num_devices=NCORES)

    eftT = nc.dram_tensor("eftT", (F, epc), dt.float16, kind="ExternalInput")
    nftsT = nc.dram_tensor("nftsT", (F, epc), dt.float16, kind="ExternalInput")
    nftdT = nc.dram_tensor("nftdT", (F, epc), dt.float16, kind="ExternalInput")
    dstT_in = nc.dram_tensor("dstT", (128, ntiles), dt.float16, kind="ExternalInput")
    nftT_own = nc.dram_tensor("nftT_own", (F, NPC), dt.float16, kind="ExternalInput")
    nftres_in = nc.dram_tensor("nftres", (NPC, F), dt.float16, kind="ExternalInput")
    w1qa_in = nc.dram_tensor("W1qa", (F, F2), dt.float16, kind="ExternalInput")
    w2cat_in = nc.dram_tensor("W2cat", (F, F2), dt.float16, kind="ExternalInput")
    w3a2_in = nc.dram_tensor("W3A2", (F, H), dt.float16, kind="ExternalInput")
    w3_in = nc.dram_tensor("W3", (F, F), dt.float16, kind="ExternalInput")
    if has_bias:
        brow_in = nc.dram_tensor("brow", (1, F2), dt.float16, kind="ExternalInput")

    out_d = nc.dram_tensor("out", (NPC, F), dt.float32, kind="ExternalOutput")

    with TileContext(nc) as tc:
        with tc.tile_pool(name="const", bufs=1) as cpool, \
             tc.tile_pool(name="work", bufs=3) as pool, \
             tc.tile_pool(name="io", bufs=6) as iop, \
             tc.tile_pool(name="psM", bufs=4, space="PSUM") as psM, \
             tc.tile_pool(name="psB", bufs=2, space="PSUM") as psB:

            # ---------- constants ----------
            iotaM = cpool.tile([128, 4, 128], dt.float32)
            nc.gpsimd.iota(iotaM, pattern=[[0, 4], [1, 128]],
                           channel_multiplier=0,
                           allow_small_or_imprecise_dtypes=True)
            iotaM16 = cpool.tile([128, 4, 128], dt.float16)
            nc.vector.tensor_copy(out=iotaM16, in_=iotaM)
            nshift = cpool.tile([128, 1], dt.float32)
            nc.vector.memset(nshift, -EXP_SHIFT)

            w1qa_s = cpool.tile([F, F2], dt.float16)
            nc.sync.dma_start(out=w1qa_s, in_=w1qa_in[:, :])
            w2cat_s = cpool.tile([F, F2], dt.float16)
            nc.sync.dma_start(out=w2cat_s, in_=w2cat_in[:, :])
            w3a2_s = cpool.tile([F, H], dt.float16)
            nc.sync.dma_start(out=w3a2_s, in_=w3a2_in[:, :])
            w3_s = cpool.tile([F, F], dt.float16)
            nc.sync.dma_start(out=w3_s, in_=w3_in[:, :])

            dstT_s = cpool.tile([128, ntiles], dt.float16)
            nc.sync.dma_start(out=dstT_s, in_=dstT_in[:, :])
            nftT_own_s = cpool.tile([F, NPC], dt.float16)
            nc.sync.dma_start(out=nftT_own_s, in_=nftT_own[:, :])
            nftres_s = cpool.tile([128, BPC, F], dt.float16)
            nc.sync.dma_start(out=nftres_s,
                              in_=nftres_in[:, :].rearrange("(b p) c -> p b c", p=128))

            if has_bias:
                brow_s1 = cpool.tile([1, F2], dt.float16)
                nc.sync.dma_start(out=brow_s1, in_=brow_in[:, :])
                ones_col = cpool.tile([1, 128], dt.float16)
                nc.vector.memset(ones_col, 1.0)
                psbb = psM.tile([128, F2], dt.float32, tag="bb")
                nc.tensor.matmul(psbb, lhsT=ones_col, rhs=brow_s1,
                                 start=True, stop=True)
                brow_bc = cpool.tile([128, F2], dt.float32)
                nc.vector.tensor_copy(out=brow_bc, in_=psbb)

            # ---------- phase 1: per-block y3 = nft_own @ W3 ----------
            y3tab = cpool.tile([128, BPC, F], dt.float16)
            for b in range(BPC):
                psy = psB.tile([128, F], dt.float32, tag="y3")
                nc.tensor.matmul(psy, lhsT=nftT_own_s[:, b * 128:(b + 1) * 128],
                                 rhs=w3_s, start=True, stop=True)
                if has_bias:
                    nc.vector.tensor_tensor(out=y3tab[:, b, :], in0=psy,
                                            in1=brow_bc[:, 0:F], op=AOP.add)
                else:
                    nc.scalar.activation(y3tab[:, b, :], psy,
                                         mybir.ActivationFunctionType.Copy)

            # ---------- phase 2 + 3 ----------
            for g in range(BPC):
                ew = tpb * 128  # edges per block
                eft_ch = iop.tile([128, ew], dt.float16, tag="eft")
                nc.sync.dma_start(out=eft_ch, in_=eftT[:, g * ew:(g + 1) * ew])
                nfts_ch = iop.tile([128, ew], dt.float16, tag="nfts")
                nc.sync.dma_start(out=nfts_ch, in_=nftsT[:, g * ew:(g + 1) * ew])
                nftd_ch = iop.tile([128, ew], dt.float16, tag="nftd")
                nc.sync.dma_start(out=nftd_ch, in_=nftdT[:, g * ew:(g + 1) * ew])

                psb = psB.tile([128, F2], dt.float32, tag="agg")
                n4 = (tpb + 3) // 4
                for t4 in range(n4):
                    k4 = min(4, tpb - t4 * 4)
                    c0 = g * tpb + t4 * 4
                    P4 = pool.tile([128, 4, 128], dt.float16, tag="P4")
                    nc.vector.tensor_tensor(
                        out=P4[:, 0:k4, :], in0=iotaM16[:, 0:k4, :],
                        in1=dstT_s[:, c0:c0 + k4][:, :, None]
                            .broadcast_to((128, k4, 128)),
                        op=AOP.is_equal)
                    msgu4 = pool.tile([128, 4, F2], dt.float16, tag="msgu4")
                    for k in range(k4):
                        t = t4 * 4 + k
                        e0 = t * 128
                        psm = psM.tile([128, F2], dt.float32, tag="m")
                        nc.tensor.matmul(psm[:, 0:F2],
                                         lhsT=eft_ch[:, e0:e0 + 128],
                                         rhs=w2cat_s, start=True, stop=False,
                                         skip_group_check=True)
                        nc.tensor.matmul(psm[:, 0:F2],
                                         lhsT=nfts_ch[:, e0:e0 + 128],
                                         rhs=w1qa_s, start=False, stop=False,
                                         skip_group_check=True)
                        nc.tensor.matmul(psm[:, F:F2],
                                         lhsT=nftd_ch[:, e0:e0 + 128],
                                         rhs=w3a2_s, start=False, stop=True,
                                         skip_group_check=True)
                        a4 = pool.tile([128, H], dt.float32, tag="a4")
                        z4 = pool.tile([128, H], dt.float32, tag="z4")
                        if has_bias:
                            nc.vector.tensor_tensor(out=z4, in0=psm[:, F:F2],
                                                    in1=brow_bc[:, F:F2],
                                                    op=AOP.add)
                        else:
                            nc.vector.tensor_copy(out=z4, in_=psm[:, F:F2])
                        nc.vector.scalar_tensor_tensor(
                            out=a4, in0=z4, scalar=0.01, in1=z4,
                            op0=AOP.mult, op1=AOP.max)
                        nc.scalar.activation(msgu4[:, k, F:F2], a4,
                                             mybir.ActivationFunctionType.Exp,
                                             bias=nshift[:, :])
                        if has_bias:
                            pb = pool.tile([128, F], dt.float32, tag="pb")
                            nc.vector.tensor_tensor(out=pb, in0=psm[:, 0:F],
                                                    in1=brow_bc[:, 0:F],
                                                    op=AOP.add)
                            src_part = pb
                        else:
                            src_part = psm[:, 0:F]
                        nc.vector.tensor_tensor(
                            out=msgu4[:, k, 0:F].rearrange("p (h d) -> p h d", h=H),
                            in0=src_part.rearrange("p (h d) -> p h d", h=H),
                            in1=msgu4[:, k, F:F2][:, :, None]
                                .broadcast_to((128, H, DH)),
                            op=AOP.mult)
                        nc.tensor.matmul(psb, lhsT=P4[:, k, :],
                                         rhs=msgu4[:, k, :],
                                         start=(t == 0), stop=(t == tpb - 1),
                                         skip_group_check=True)

                # ---------- phase 3 for block g ----------
                ss = pool.tile([128, H], dt.float32, tag="ss")
                nc.vector.tensor_scalar(out=ss, in0=psb[:, F:F2],
                                        scalar1=1e-30, scalar2=None,
                                        op0=AOP.max)
                inv = pool.tile([128, H], dt.float32, tag="inv")
                nc.vector.reciprocal(inv, ss)
                red = pool.tile([128, 1], dt.float32, tag="red")
                nc.vector.reduce_sum(red, psb[:, F:F2],
                                     axis=mybir.AxisListType.X)
                msk = pool.tile([128, 1], dt.float32, tag="msk")
                nc.vector.tensor_scalar(out=msk, in0=red, scalar1=0.0,
                                        scalar2=None, op0=AOP.is_gt)
                y3m = pool.tile([128, F], dt.float32, tag="y3m")
                nc.vector.tensor_tensor(out=y3m, in0=y3tab[:, g, :],
                                        in1=msk[:, :].broadcast_to((128, F)),
                                        op=AOP.mult)
                mn = pool.tile([128, F], dt.float32, tag="mn")
                nc.vector.tensor_tensor(
                    out=mn[:, :].rearrange("p (h d) -> p h d", h=H),
                    in0=psb[:, 0:F].rearrange("p (h d) -> p h d", h=H),
                    in1=inv[:, :, None].broadcast_to((128, H, DH)),
                    op=AOP.mult)
                s1 = pool.tile([128, F], dt.float32, tag="s1")
                nc.vector.tensor_tensor(out=s1, in0=mn, in1=y3m, op=AOP.add)
                s2 = pool.tile([128, F], dt.float32, tag="s2")
                nc.vector.tensor_tensor(out=s2, in0=s1, in1=nftres_s[:, g, :],
                                        op=AOP.add)
                oc = pool.tile([128, F], dt.float32, tag="oc")
                nc.scalar.activation(oc, s2, mybir.ActivationFunctionType.Relu)
                nc.sync.dma_start(out=out_d[g * 128:(g + 1) * 128, :], in_=oc)

    nc.compile()
    return nc


def pack_nodes(dst, n_nodes, edge_cap):
    """LPT bin-packing of nodes into NBLK blocks (<=NODE_CAP nodes,
    <=edge_cap in-edges). Returns (block_of_node, slot_of_node, nodes_per
    [NBLK, NODE_CAP] with -1 padding) or None if infeasible."""
    import heapq
    deg = np.bincount(dst, minlength=n_nodes).astype(np.int64)
    order = np.argsort(-deg, kind="stable")
    heap = [(0, b) for b in range(NBLK)]
    heapq.heapify(heap)
    counts = np.zeros(NBLK, dtype=np.int64)
    loads = np.zeros(NBLK, dtype=np.int64)
    block_of = np.empty(n_nodes, dtype=np.int32)
    slot_of = np.empty(n_nodes, dtype=np.int32)
    spill = []
    for n in order:
        d = int(deg[n])
        placed = False
        while heap:
            load, b = heapq.heappop(heap)
            if counts[b] >= NODE_CAP:
                continue  # node-full: drop from heap permanently
            if load + d <= edge_cap:
                block_of[n] = b
                slot_of[n] = counts[b]
                counts[b] += 1
                loads[b] = load + d
                heapq.heappush(heap, (loads[b], b))
                placed = True
                break
            else:
                spill.append((load, b))
        for item in spill:
            heapq.heappush(heap, item)
        spill.clear()
        if not placed:
            return None
    nodes_per = np.full((NBLK, NODE_CAP), -1, dtype=np.int64)
    nodes_per[block_of, slot_of] = np.arange(n_nodes)
    return block_of, slot_of, nodes_per


def prep_inputs(nft, eft, W_path, b_path, W_attn1, attn2, src, dst, tpb,
                block_of, slot_of, nodes_per):
    n_nodes = nft.shape[0]
    edge_cap = tpb * 128
    epc = BPC * edge_cap
    E = eft.shape[0]

    nft16 = nft.astype(np.float16)
    eft16 = eft.astype(np.float16)

    # global edge slotting: position = block*edge_cap + rank_within_block
    eb = block_of[dst]
    eorder = np.argsort(eb, kind="stable")
    counts = np.bincount(eb, minlength=NBLK)
    assert counts.max() <= edge_cap
    starts = np.zeros(NBLK + 1, dtype=np.int64)
    np.cumsum(counts, out=starts[1:])
    rank = np.arange(E, dtype=np.int64) - starts[eb[eorder]]
    pos = eb[eorder] * edge_cap + rank          # slot for edge eorder[i]

    TOT = NBLK * edge_cap
    eft_all = np.zeros((TOT, F), dtype=np.float16)
    nfts_all = np.zeros((TOT, F), dtype=np.float16)
    nftd_all = np.zeros((TOT, F), dtype=np.float16)
    dstloc_all = np.full(TOT, DUMMY_SLOT, dtype=np.float16)
    es = eorder
    eft_all[pos] = eft16[es]
    nfts_all[pos] = nft16[src[es]]
    nftd_all[pos] = nft16[dst[es]]
    dstloc_all[pos] = slot_of[dst[es]].astype(np.float16)

    # weights (host-side pure weight algebra, like the baseline's A2blk)
    a2 = np.asarray(attn2, dtype=np.float32).reshape(H, DH)
    A2blk = np.zeros((F, H), dtype=np.float32)
    for h in range(H):
        A2blk[h * DH:(h + 1) * DH, h] = a2[h]
    Wp = np.asarray(W_path, dtype=np.float32)
    W1, W2, W3 = Wp[0:F], Wp[F:2 * F], Wp[2 * F:3 * F]
    W1qa = np.concatenate([W1, W1 @ A2blk + np.asarray(W_attn1, np.float32)],
                          axis=1).astype(np.float16)
    W2cat = np.concatenate([W2, W2 @ A2blk], axis=1).astype(np.float16)
    W3A2 = (W3 @ A2blk).astype(np.float16)
    W3_16 = W3.astype(np.float16)

    b = np.asarray(b_path, dtype=np.float32).reshape(F)
    has_bias = bool(np.any(b != 0))
    brow = np.concatenate([b, b @ A2blk]).reshape(1, F2).astype(np.float16)

    in_maps = []
    for c in range(NCORES):
        lo = c * BPC * edge_cap
        hi = lo + epc
        own = nodes_per[c * BPC:(c + 1) * BPC].reshape(-1)  # [NPC] node ids
        ok = own >= 0
        nftres = np.zeros((NPC, F), dtype=np.float16)
        nftres[ok] = nft16[own[ok]]
        m = {
            "eftT": np.ascontiguousarray(eft_all[lo:hi].T),
            "nftsT": np.ascontiguousarray(nfts_all[lo:hi].T),
            "nftdT": np.ascontiguousarray(nftd_all[lo:hi].T),
            "dstT": np.ascontiguousarray(
                dstloc_all[lo:hi].reshape(-1, 128).T),
            "nftT_own": np.ascontiguousarray(nftres.T),
            "nftres": nftres,
            "W1qa": W1qa,
            "W2cat": W2cat,
            "W3A2": W3A2,
            "W3": W3_16,
        }
        if has_bias:
            m["brow"] = brow
        in_maps.append(m)
    return in_maps, has_bias


_NC_CACHE = {}


def _get_nc(key, *args):
    if key not in _NC_CACHE:
        _NC_CACHE[key] = build_nc(*args)
    return _NC_CACHE[key]


def run(nft, eft, W_path, b_path, W_attn1, attn2, src, dst, trace=False,
        tmpdir=None, prec=None):
    nft = np.asarray(nft, dtype=np.float32)
    eft = np.asarray(eft, dtype=np.float32)
    src = np.asarray(src, dtype=np.int64)
    dst = np.asarray(dst, dtype=np.int64)
    n_nodes = nft.shape[0]
    assert n_nodes <= NBLK * NODE_CAP

    tpb = 16
    packed = pack_nodes(dst, n_nodes, tpb * 128)
    while packed is None:
        tpb += 1
        packed = pack_nodes(dst, n_nodes, tpb * 128)
    block_of, slot_of, nodes_per = packed

    in_maps, has_bias = prep_inputs(nft, eft, np.asarray(W_path),
                                    np.asarray(b_path), np.asarray(W_attn1),
                                    np.asarray(attn2), src, dst, tpb,
                                    block_of, slot_of, nodes_per)
    nc = _get_nc((tpb, has_bias), tpb, has_bias)
    kw = {}
    if trace:
        kw = dict(trace=True, tmpdir=tmpdir)
    res = bass_utils.run_bass_kernel_spmd(nc, in_maps,
                                          core_ids=list(range(NCORES)), **kw)

    out = np.empty((n_nodes, F), dtype=np.float32)
    for c in range(NCORES):
        own = nodes_per[c * BPC:(c + 1) * BPC].reshape(-1)
        ok = own >= 0
        out[own[ok]] = res.results[c]["out"][ok]
    return out, res


def kernel(**inputs):
    out, _ = run(**inputs)
    return out


# revision 9
# speedup vs baseline: 1.3524x; 1.3524x over previous
"""GAT message-passing kernel for 8 Trainium2 NeuronCores (Bass/Tile).

Strategy ("route edges by dst ownership", gather-free):
  - Host bin-packs nodes into 160 blocks (<=128 nodes, <=2048 in-edges each);
    each core owns 20 blocks, so segment-softmax and scatter-sum are fully
    core-local (no collectives, no device gather).
  - The host ships per-edge operand streams in tile order: eft[e].T,
    nft[src[e]].T and nft[dst[e]].T (pure index-gathers of the inputs, all
    f16).  Every FLOP of the model runs on device:
      per 128-edge tile, one PSUM tile [e,136] accumulates three matmuls:
        etT @ [W2 | W2A2]          (edge features -> eW2 and logit part)
        nftsT @ [W1 | W1A2+Wa1]    (y1[src] and qa[src])
        nftdT @ [W3A2]             (r[dst], logit columns only)
    giving part = y1+eW2 (cols 0:128) and z = qa+eW2A2+r (cols 128:136).
  - Softmax without max-subtraction: u = exp(leaky(z) - 7) (shift-invariant,
    fp16-safe); messages msgu = part*u with u riding in cols 128:136; a
    per-tile one-hot P (built on-device by is_equal against an iota) scatters
    [msg | u] into the block accumulator psb = [sum u*part | sum u].
  - Per block: mn = agg/s, + y3 (nft_own@W3, computed on device, masked to
    nodes with in-edges), + nft residual, relu, store [node, feat].
"""

import sys
import numpy as np

for _p in ("/opt/trn_rl_repo",):
    if _p not in sys.path:
        sys.path.append(_p)

import concourse.bacc as bacc
import concourse.bass as bass
import concourse.mybir as mybir
from concourse.tile import TileContext
from concourse import bass_utils

F = 128
H = 8
DH = 16
F2 = F + H           # 136
NCORES = 8
NODE_CAP = 128       # node slots per block
BPC = 20             # blocks per core
NPC = BPC * NODE_CAP # 2560 node slots per core
NBLK = NCORES * BPC  # 160 blocks
EXP_SHIFT = 7.0      # u = exp(a - shift); softmax-invariant, keeps u in f16
DUMMY_SLOT = 200.0   # dstloc for dummy edges: is_equal never matches


def build_nc(tpb, has_bias):
    """tpb: tiles (of 128 edges) per block; edge cap per block = 128*tpb."""
    ntiles = BPC * tpb
    epc = ntiles * 128
    dt = mybir.dt
    AOP = mybir.AluOpType

    nc = bacc.Bacc("TRN2", target_bir_lowering=False, debug=False,
                   num_devices=NCORES)

    eftT = nc.dram_tensor("eftT", (F, epc), dt.float16, kind="ExternalInput")
    nftsT = nc.dram_tensor("nftsT", (F, epc), dt.float16, kind="ExternalInput")
    nftdT = nc.dram_tensor("nftdT", (F, epc), dt.float16, kind="ExternalInput")
    dstT_in = nc.dram_tensor("dstT", (128, ntiles), dt.float16, kind="ExternalInput")
    nftT_own = nc.dram_tensor("nftT_own", (F, NPC), dt.float16, kind="ExternalInput")
    nftres_in = nc.dram_tensor("nftres", (NPC, F), dt.float16, kind="ExternalInput")
    w1qa_in = nc.dram_tensor("W1qa", (F, F2), dt.float16, kind="ExternalInput")
    w2cat_in = nc.dram_tensor("W2cat", (F, F2), dt.float16, kind="ExternalInput")
    w3a2_in = nc.dram_tensor("W3A2", (F, H), dt.float16, kind="ExternalInput")
    w3_in = nc.dram_tensor("W3", (F, F), dt.float16, kind="ExternalInput")
    if has_bias:
        brow_in = nc.dram_tensor("brow", (1, F2), dt.float16, kind="ExternalInput")

    out_d = nc.dram_tensor("out", (NPC, F), dt.float32, kind="ExternalOutput")

    with TileContext(nc) as tc:
        with tc.tile_pool(name="const", bufs=1) as cpool, \
             tc.tile_pool(name="work", bufs=3) as pool, \
             tc.tile_pool(name="io", bufs=6) as iop, \
             tc.tile_pool(name="psM", bufs=4, space="PSUM") as psM, \
             tc.tile_pool(name="psB", bufs=2, space="PSUM") as psB:

            # ---------- constants ----------
            iotaM = cpool.tile([128, tpb, 128], dt.float32)
            nc.gpsimd.iota(iotaM, pattern=[[0, tpb], [1, 128]],
                           channel_multiplier=0,
                           allow_small_or_imprecise_dtypes=True)
            iotaB16 = cpool.tile([128, tpb, 128], dt.float16)
            nc.vector.tensor_copy(out=iotaB16, in_=iotaM)
            nshift = cpool.tile([128, 1], dt.float32)
            nc.vector.memset(nshift, -EXP_SHIFT)

            w1qa_s = cpool.tile([F, F2], dt.float16)
            nc.sync.dma_start(out=w1qa_s, in_=w1qa_in[:, :])
            w2cat_s = cpool.tile([F, F2], dt.float16)
            nc.sync.dma_start(out=w2cat_s, in_=w2cat_in[:, :])
            w3a2_s = cpool.tile([F, H], dt.float16)
            nc.sync.dma_start(out=w3a2_s, in_=w3a2_in[:, :])
            w3_s = cpool.tile([F, F], dt.float16)
            nc.sync.dma_start(out=w3_s, in_=w3_in[:, :])

            dstT_s = cpool.tile([128, ntiles], dt.float16)
            nc.sync.dma_start(out=dstT_s, in_=dstT_in[:, :])
            nftT_own_s = cpool.tile([F, NPC], dt.float16)
            nc.sync.dma_start(out=nftT_own_s, in_=nftT_own[:, :])
            nftres_s = cpool.tile([128, BPC, F], dt.float16)
            nc.sync.dma_start(out=nftres_s,
                              in_=nftres_in[:, :].rearrange("(b p) c -> p b c", p=128))

            if has_bias:
                brow_s1 = cpool.tile([1, F2], dt.float16)
                nc.sync.dma_start(out=brow_s1, in_=brow_in[:, :])
                ones_col = cpool.tile([1, 128], dt.float16)
                nc.vector.memset(ones_col, 1.0)
                psbb = psM.tile([128, F2], dt.float32, tag="bb")
                nc.tensor.matmul(psbb, lhsT=ones_col, rhs=brow_s1,
                                 start=True, stop=True)
                brow_bc = cpool.tile([128, F2], dt.float32)
                nc.vector.tensor_copy(out=brow_bc, in_=psbb)

            # ---------- phase 1: per-block y3 = nft_own @ W3 ----------
            y3tab = cpool.tile([128, BPC, F], dt.float16)
            for b in range(BPC):
                psy = psB.tile([128, F], dt.float32, tag="y3")
                nc.tensor.matmul(psy, lhsT=nftT_own_s[:, b * 128:(b + 1) * 128],
                                 rhs=w3_s, start=True, stop=True)
                if has_bias:
                    nc.vector.tensor_tensor(out=y3tab[:, b, :], in0=psy,
                                            in1=brow_bc[:, 0:F], op=AOP.add)
                else:
                    nc.scalar.activation(y3tab[:, b, :], psy,
                                         mybir.ActivationFunctionType.Copy)

            # ---------- phase 2 + 3 ----------
            for g in range(BPC):
                ew = tpb * 128  # edges per block
                eft_ch = iop.tile([128, ew], dt.float16, tag="eft")
                nc.sync.dma_start(out=eft_ch, in_=eftT[:, g * ew:(g + 1) * ew])
                nfts_ch = iop.tile([128, ew], dt.float16, tag="nfts")
                nc.sync.dma_start(out=nfts_ch, in_=nftsT[:, g * ew:(g + 1) * ew])
                nftd_ch = iop.tile([128, ew], dt.float16, tag="nftd")
                nc.sync.dma_start(out=nftd_ch, in_=nftdT[:, g * ew:(g + 1) * ew])

                psb = psB.tile([128, F2], dt.float32, tag="agg")
                Pb = pool.tile([128, tpb, 128], dt.float16, tag="Pb")
                nc.vector.tensor_tensor(
                    out=Pb, in0=iotaB16,
                    in1=dstT_s[:, g * tpb:(g + 1) * tpb][:, :, None]
                        .broadcast_to((128, tpb, 128)),
                    op=AOP.is_equal)
                ngrp = (tpb + 2) // 3
                for t3 in range(ngrp):
                    k3 = min(3, tpb - t3 * 3)
                    psm = psM.tile([128, 3, F2], dt.float32, tag="m")
                    msgu = pool.tile([128, 3, F2], dt.float16, tag="msgu")
                    for k in range(k3):
                        t = t3 * 3 + k
                        e0 = t * 128
                        nc.tensor.matmul(psm[:, k, 0:F2],
                                         lhsT=eft_ch[:, e0:e0 + 128],
                                         rhs=w2cat_s, start=True, stop=False,
                                         skip_group_check=True)
                        nc.tensor.matmul(psm[:, k, 0:F2],
                                         lhsT=nfts_ch[:, e0:e0 + 128],
                                         rhs=w1qa_s, start=False, stop=False,
                                         skip_group_check=True)
                        nc.tensor.matmul(psm[:, k, F:F2],
                                         lhsT=nftd_ch[:, e0:e0 + 128],
                                         rhs=w3a2_s, start=False, stop=True,
                                         skip_group_check=True)
                    a4 = pool.tile([128, 3, H], dt.float32, tag="a4")
                    z4 = pool.tile([128, 3, H], dt.float32, tag="z4")
                    if has_bias:
                        nc.vector.tensor_tensor(
                            out=z4[:, 0:k3, :], in0=psm[:, 0:k3, F:F2],
                            in1=brow_bc[:, F:F2][:, None, :]
                                .broadcast_to((128, k3, H)),
                            op=AOP.add)
                    else:
                        nc.vector.tensor_copy(out=z4[:, 0:k3, :],
                                              in_=psm[:, 0:k3, F:F2])
                    nc.vector.scalar_tensor_tensor(
                        out=a4[:, 0:k3, :], in0=z4[:, 0:k3, :], scalar=0.01,
                        in1=z4[:, 0:k3, :], op0=AOP.mult, op1=AOP.max)
                    nc.scalar.activation(msgu[:, 0:k3, F:F2], a4[:, 0:k3, :],
                                         mybir.ActivationFunctionType.Exp,
                                         bias=nshift[:, :])
                    if has_bias:
                        pb = pool.tile([128, 3, F], dt.float32, tag="pb")
                        nc.vector.tensor_tensor(
                            out=pb[:, 0:k3, :], in0=psm[:, 0:k3, 0:F],
                            in1=brow_bc[:, 0:F][:, None, :]
                                .broadcast_to((128, k3, F)),
                            op=AOP.add)
                        src_part = pb[:, 0:k3, 0:F]
                    else:
                        src_part = psm[:, 0:k3, 0:F]
                    nc.vector.tensor_tensor(
                        out=msgu[:, 0:k3, 0:F].rearrange(
                            "p k (h d) -> p k h d", h=H),
                        in0=src_part.rearrange("p k (h d) -> p k h d", h=H),
                        in1=msgu[:, 0:k3, F:F2][:, :, :, None]
                            .broadcast_to((128, k3, H, DH)),
                        op=AOP.mult)
                    for k in range(k3):
                        t = t3 * 3 + k
                        nc.tensor.matmul(psb, lhsT=Pb[:, t, :],
                                         rhs=msgu[:, k, :],
                                         start=(t == 0), stop=(t == tpb - 1),
                                         skip_group_check=True)

                # ---------- phase 3 for block g ----------
                ss = pool.tile([128, H], dt.float32, tag="ss")
                nc.vector.tensor_scalar(out=ss, in0=psb[:, F:F2],
                                        scalar1=1e-30, scalar2=None,
                                        op0=AOP.max)
                inv = pool.tile([128, H], dt.float32, tag="inv")
                nc.vector.reciprocal(inv, ss)
                red = pool.tile([128, 1], dt.float32, tag="red")
                nc.vector.reduce_sum(red, psb[:, F:F2],
                                     axis=mybir.AxisListType.X)
                msk = pool.tile([128, 1], dt.float32, tag="msk")
                nc.vector.tensor_scalar(out=msk, in0=red, scalar1=0.0,
                                        scalar2=None, op0=AOP.is_gt)
                mn = pool.tile([128, F], dt.float32, tag="mn")
                nc.vector.tensor_tensor(
                    out=mn[:, :].rearrange("p (h d) -> p h d", h=H),
                    in0=psb[:, 0:F].rearrange("p (h d) -> p h d", h=H),
                    in1=inv[:, :, None].broadcast_to((128, H, DH)),
                    op=AOP.mult)
                # s1 = y3*msk + mn, s2 = nftres + s1
                s1 = pool.tile([128, F], dt.float32, tag="s1")
                nc.vector.scalar_tensor_tensor(
                    out=s1, in0=y3tab[:, g, :], scalar=msk[:, :], in1=mn,
                    op0=AOP.mult, op1=AOP.add)
                s2 = pool.tile([128, F], dt.float32, tag="s2")
                nc.vector.tensor_tensor(out=s2, in0=s1, in1=nftres_s[:, g, :],
                                        op=AOP.add)
                oc = pool.tile([128, F], dt.float32, tag="oc")
                nc.scalar.activation(oc, s2, mybir.ActivationFunctionType.Relu)
                nc.sync.dma_start(out=out_d[g * 128:(g + 1) * 128, :], in_=oc)

    nc.compile()
    return nc


def pack_nodes(dst, n_nodes, edge_cap):
    """LPT bin-packing of nodes into NBLK blocks (<=NODE_CAP nodes,
    <=edge_cap in-edges). Returns (block_of_node, slot_of_node, nodes_per
    [NBLK, NODE_CAP] with -1 padding) or None if infeasible."""
    import heapq
    deg = np.bincount(dst, minlength=n_nodes).astype(np.int64)
    order = np.argsort(-deg, kind="stable")
    heap = [(0, b) for b in range(NBLK)]
    heapq.heapify(heap)
    counts = np.zeros(NBLK, dtype=np.int64)
    loads = np.zeros(NBLK, dtype=np.int64)
    block_of = np.empty(n_nodes, dtype=np.int32)
    slot_of = np.empty(n_nodes, dtype=np.int32)
    spill = []
    for n in order:
        d = int(deg[n])
        placed = False
        while heap:
            load, b = heapq.heappop(heap)
            if counts[b] >= NODE_CAP:
                continue  # node-full: drop from heap permanently
            if load + d <= edge_cap:
                block_of[n] = b
                slot_of[n] = counts[b]
                counts[b] += 1
                loads[b] = load + d
                heapq.heappush(heap, (loads[b], b))
                placed = True
                break
            else:
                spill.append((load, b))
        for item in spill:
            heapq.heappush(heap, item)
        spill.clear()
        if not placed:
            return None
    nodes_per = np.full((NBLK, NODE_CAP), -1, dtype=np.int64)
    nodes_per[block_of, slot_of] = np.arange(n_nodes)
    return block_of, slot_of, nodes_per


def prep_inputs(nft, eft, W_path, b_path, W_attn1, attn2, src, dst, tpb,
                block_of, slot_of, nodes_per):
    n_nodes = nft.shape[0]
    edge_cap = tpb * 128
    epc = BPC * edge_cap
    E = eft.shape[0]

    nft16 = nft.astype(np.float16)
    eft16 = eft.astype(np.float16)

    # global edge slotting: position = block*edge_cap + rank_within_block
    eb = block_of[dst]
    eorder = np.argsort(eb, kind="stable")
    counts = np.bincount(eb, minlength=NBLK)
    assert counts.max() <= edge_cap
    starts = np.zeros(NBLK + 1, dtype=np.int64)
    np.cumsum(counts, out=starts[1:])
    rank = np.arange(E, dtype=np.int64) - starts[eb[eorder]]
    pos = eb[eorder] * edge_cap + rank          # slot for edge eorder[i]

    TOT = NBLK * edge_cap
    eft_all = np.zeros((TOT, F), dtype=np.float16)
    nfts_all = np.zeros((TOT, F), dtype=np.float16)
    nftd_all = np.zeros((TOT, F), dtype=np.float16)
    dstloc_all = np.full(TOT, DUMMY_SLOT, dtype=np.float16)
    es = eorder
    eft_all[pos] = eft16[es]
    nfts_all[pos] = nft16[src[es]]
    nftd_all[pos] = nft16[dst[es]]
    dstloc_all[pos] = slot_of[dst[es]].astype(np.float16)

    # weights (host-side pure weight algebra, like the baseline's A2blk)
    a2 = np.asarray(attn2, dtype=np.float32).reshape(H, DH)
    A2blk = np.zeros((F, H), dtype=np.float32)
    for h in range(H):
        A2blk[h * DH:(h + 1) * DH, h] = a2[h]
    Wp = np.asarray(W_path, dtype=np.float32)
    W1, W2, W3 = Wp[0:F], Wp[F:2 * F], Wp[2 * F:3 * F]
    W1qa = np.concatenate([W1, W1 @ A2blk + np.asarray(W_attn1, np.float32)],
                          axis=1).astype(np.float16)
    W2cat = np.concatenate([W2, W2 @ A2blk], axis=1).astype(np.float16)
    W3A2 = (W3 @ A2blk).astype(np.float16)
    W3_16 = W3.astype(np.float16)

    b = np.asarray(b_path, dtype=np.float32).reshape(F)
    has_bias = bool(np.any(b != 0))
    brow = np.concatenate([b, b @ A2blk]).reshape(1, F2).astype(np.float16)

    in_maps = []
    for c in range(NCORES):
        lo = c * BPC * edge_cap
        hi = lo + epc
        own = nodes_per[c * BPC:(c + 1) * BPC].reshape(-1)  # [NPC] node ids
        ok = own >= 0
        nftres = np.zeros((NPC, F), dtype=np.float16)
        nftres[ok] = nft16[own[ok]]
        m = {
            "eftT": np.ascontiguousarray(eft_all[lo:hi].T),
            "nftsT": np.ascontiguousarray(nfts_all[lo:hi].T),
            "nftdT": np.ascontiguousarray(nftd_all[lo:hi].T),
            "dstT": np.ascontiguousarray(
                dstloc_all[lo:hi].reshape(-1, 128).T),
            "nftT_own": np.ascontiguousarray(nftres.T),
            "nftres": nftres,
            "W1qa": W1qa,
            "W2cat": W2cat,
            "W3A2": W3A2,
            "W3": W3_16,
        }
        if has_bias:
            m["brow"] = brow
        in_maps.append(m)
    return in_maps, has_bias


_NC_CACHE = {}


def _get_nc(key, *args):
    if key not in _NC_CACHE:
        _NC_CACHE[key] = build_nc(*args)
    return _NC_CACHE[key]


def run(nft, eft, W_path, b_path, W_attn1, attn2, src, dst, trace=False,
        tmpdir=None, prec=None):
    nft = np.asarray(nft, dtype=np.float32)
    eft = np.asarray(eft, dtype=np.float32)
    src = np.asarray(src, dtype=np.int64)
    dst = np.asarray(dst, dtype=np.int64)
    n_nodes = nft.shape[0]
    assert n_nodes <= NBLK * NODE_CAP

    tpb = 16
    packed = pack_nodes(dst, n_nodes, tpb * 128)
    while packed is None:
        tpb += 1
        packed = pack_nodes(dst, n_nodes, tpb * 128)
    block_of, slot_of, nodes_per = packed

    in_maps, has_bias = prep_inputs(nft, eft, np.asarray(W_path),
                                    np.asarray(b_path), np.asarray(W_attn1),
                                    np.asarray(attn2), src, dst, tpb,
                                    block_of, slot_of, nodes_per)
    nc = _get_nc((tpb, has_bias), tpb, has_bias)
    kw = {}
    if trace:
        kw = dict(trace=True, tmpdir=tmpdir)
    res = bass_utils.run_bass_kernel_spmd(nc, in_maps,
                                          core_ids=list(range(NCORES)), **kw)

    out = np.empty((n_nodes, F), dtype=np.float32)
    for c in range(NCORES):
        own = nodes_per[c * BPC:(c + 1) * BPC].reshape(-1)
        ok = own >= 0
        out[own[ok]] = res.results[c]["out"][ok]
    return out, res


def kernel(**inputs):
    out, _ = run(**inputs)
    return out


# revision 11
# speedup vs baseline: 1.3985x; 1.0341x over previous
"""GAT message-passing kernel for 8 Trainium2 NeuronCores (Bass/Tile).

Strategy ("route edges by dst ownership", gather-free):
  - Host bin-packs nodes into 160 blocks (<=128 nodes, <=2048 in-edges each);
    each core owns 20 blocks, so segment-softmax and scatter-sum are fully
    core-local (no collectives, no device gather).
  - The host ships per-edge operand streams in tile order: eft[e].T,
    nft[src[e]].T and nft[dst[e]].T (pure index-gathers of the inputs, all
    f16).  Every FLOP of the model runs on device:
      per 128-edge tile, one PSUM tile [e,136] accumulates three matmuls:
        etT @ [W2 | W2A2]          (edge features -> eW2 and logit part)
        nftsT @ [W1 | W1A2+Wa1]    (y1[src] and qa[src])
        nftdT @ [W3A2]             (r[dst], logit columns only)
    giving part = y1+eW2 (cols 0:128) and z = qa+eW2A2+r (cols 128:136).
  - Softmax without max-subtraction: u = exp(leaky(z) - 7) (shift-invariant,
    fp16-safe); messages msgu = part*u with u riding in cols 128:136; a
    per-tile one-hot P (built on-device by is_equal against an iota) scatters
    [msg | u] into the block accumulator psb = [sum u*part | sum u].
  - Per block: mn = agg/s, + y3 (nft_own@W3, computed on device, masked to
    nodes with in-edges), + nft residual, relu, store [node, feat].
"""

import sys
import numpy as np

for _p in ("/opt/trn_rl_repo",):
    if _p not in sys.path:
        sys.path.append(_p)

import concourse.bacc as bacc
import concourse.bass as bass
import concourse.mybir as mybir
from concourse.tile import TileContext
from concourse import bass_utils

F = 128
H = 8
DH = 16
F2 = F + H           # 136
NCORES = 8
NODE_CAP = 128       # node slots per block
BPC = 20             # blocks per core
NPC = BPC * NODE_CAP # 2560 node slots per core
NBLK = NCORES * BPC  # 160 blocks
EXP_SHIFT = 7.0      # u = exp(a - shift); softmax-invariant, keeps u in f16
DUMMY_SLOT = 200.0   # dstloc for dummy edges: is_equal never matches


def build_nc(tpb, has_bias):
    """tpb: tiles (of 128 edges) per block; edge cap per block = 128*tpb."""
    ntiles = BPC * tpb
    epc = ntiles * 128
    dt = mybir.dt
    AOP = mybir.AluOpType

    nc = bacc.Bacc("TRN2", target_bir_lowering=False, debug=False,
                   num_devices=NCORES)

    eftT = nc.dram_tensor("eftT", (F, epc), dt.float16, kind="ExternalInput")
    nftsT = nc.dram_tensor("nftsT", (F, epc), dt.float16, kind="ExternalInput")
    nftdT = nc.dram_tensor("nftdT", (F, epc), dt.float16, kind="ExternalInput")
    dstT_in = nc.dram_tensor("dstT", (128, ntiles), dt.float16, kind="ExternalInput")
    nftT_own = nc.dram_tensor("nftT_own", (F, NPC), dt.float16, kind="ExternalInput")
    nftres_in = nc.dram_tensor("nftres", (NPC, F), dt.float16, kind="ExternalInput")
    w1qa_in = nc.dram_tensor("W1qa", (F, F2), dt.float16, kind="ExternalInput")
    w2cat_in = nc.dram_tensor("W2cat", (F, F2), dt.float16, kind="ExternalInput")
    w3a2_in = nc.dram_tensor("W3A2", (F, H), dt.float16, kind="ExternalInput")
    w3_in = nc.dram_tensor("W3", (F, F), dt.float16, kind="ExternalInput")
    if has_bias:
        brow_in = nc.dram_tensor("brow", (1, F2), dt.float16, kind="ExternalInput")

    out_d = nc.dram_tensor("out", (NPC, F), dt.float32, kind="ExternalOutput")

    with TileContext(nc) as tc:
        with tc.tile_pool(name="const", bufs=1) as cpool, \
             tc.tile_pool(name="work", bufs=3) as pool, \
             tc.tile_pool(name="io", bufs=6) as iop, \
             tc.tile_pool(name="psM", bufs=4, space="PSUM") as psM, \
             tc.tile_pool(name="psB", bufs=2, space="PSUM") as psB:

            # ---------- constants ----------
            iotaB16 = cpool.tile([128, tpb, 128], dt.float16)
            nc.gpsimd.iota(iotaB16, pattern=[[0, tpb], [1, 128]],
                           channel_multiplier=0,
                           allow_small_or_imprecise_dtypes=True)
            nshift = cpool.tile([128, 1], dt.float32)
            nc.vector.memset(nshift, -EXP_SHIFT)

            w1qa_s = cpool.tile([F, F2], dt.float16)
            nc.sync.dma_start(out=w1qa_s, in_=w1qa_in[:, :])
            w2cat_s = cpool.tile([F, F2], dt.float16)
            nc.sync.dma_start(out=w2cat_s, in_=w2cat_in[:, :])
            w3a2_s = cpool.tile([F, H], dt.float16)
            nc.sync.dma_start(out=w3a2_s, in_=w3a2_in[:, :])
            w3_s = cpool.tile([F, F], dt.float16)
            nc.sync.dma_start(out=w3_s, in_=w3_in[:, :])

            dstT_s = cpool.tile([128, ntiles], dt.float16)
            nc.sync.dma_start(out=dstT_s, in_=dstT_in[:, :])
            nftT_own_s = cpool.tile([F, NPC], dt.float16)
            nc.sync.dma_start(out=nftT_own_s, in_=nftT_own[:, :])
            nftres_s = cpool.tile([128, BPC, F], dt.float16)
            nc.sync.dma_start(out=nftres_s,
                              in_=nftres_in[:, :].rearrange("(b p) c -> p b c", p=128))

            if has_bias:
                brow_s1 = cpool.tile([1, F2], dt.float16)
                nc.sync.dma_start(out=brow_s1, in_=brow_in[:, :])
                ones_col = cpool.tile([1, 128], dt.float16)
                nc.vector.memset(ones_col, 1.0)
                psbb = psM.tile([128, F2], dt.float32, tag="bb")
                nc.tensor.matmul(psbb, lhsT=ones_col, rhs=brow_s1,
                                 start=True, stop=True)
                brow_bc = cpool.tile([128, F2], dt.float32)
                nc.vector.tensor_copy(out=brow_bc, in_=psbb)

            # ---------- phase 1: per-block y3 = nft_own @ W3 ----------
            y3tab = cpool.tile([128, BPC, F], dt.float16)
            for b in range(BPC):
                psy = psB.tile([128, F], dt.float32, tag="y3")
                nc.tensor.matmul(psy, lhsT=nftT_own_s[:, b * 128:(b + 1) * 128],
                                 rhs=w3_s, start=True, stop=True)
                if has_bias:
                    nc.vector.tensor_tensor(out=y3tab[:, b, :], in0=psy,
                                            in1=brow_bc[:, 0:F], op=AOP.add)
                else:
                    nc.scalar.activation(y3tab[:, b, :], psy,
                                         mybir.ActivationFunctionType.Copy)

            # ---------- phase 2 + 3 ----------
            for g in range(BPC):
                ew = tpb * 128  # edges per block
                eft_ch = iop.tile([128, ew], dt.float16, tag="eft")
                nc.sync.dma_start(out=eft_ch, in_=eftT[:, g * ew:(g + 1) * ew])
                nfts_ch = iop.tile([128, ew], dt.float16, tag="nfts")
                nc.sync.dma_start(out=nfts_ch, in_=nftsT[:, g * ew:(g + 1) * ew])
                nftd_ch = iop.tile([128, ew], dt.float16, tag="nftd")
                nc.sync.dma_start(out=nftd_ch, in_=nftdT[:, g * ew:(g + 1) * ew])

                psb = psB.tile([128, F2], dt.float32, tag="agg")
                Pb = pool.tile([128, tpb, 128], dt.float16, tag="Pb")
                nc.vector.tensor_tensor(
                    out=Pb, in0=iotaB16,
                    in1=dstT_s[:, g * tpb:(g + 1) * tpb][:, :, None]
                        .broadcast_to((128, tpb, 128)),
                    op=AOP.is_equal)
                ngrp = (tpb + 2) // 3
                for t3 in range(ngrp):
                    k3 = min(3, tpb - t3 * 3)
                    psm = psM.tile([128, 3, F2], dt.float32, tag="m")
                    msgu = pool.tile([128, 3, F2], dt.float16, tag="msgu")
                    for k in range(k3):
                        t = t3 * 3 + k
                        e0 = t * 128
                        nc.tensor.matmul(psm[:, k, 0:F2],
                                         lhsT=eft_ch[:, e0:e0 + 128],
                                         rhs=w2cat_s, start=True, stop=False,
                                         skip_group_check=True)
                        nc.tensor.matmul(psm[:, k, 0:F2],
                                         lhsT=nfts_ch[:, e0:e0 + 128],
                                         rhs=w1qa_s, start=False, stop=False,
                                         skip_group_check=True)
                        nc.tensor.matmul(psm[:, k, F:F2],
                                         lhsT=nftd_ch[:, e0:e0 + 128],
                                         rhs=w3a2_s, start=False, stop=True,
                                         skip_group_check=True)
                    a4 = pool.tile([128, 3, H], dt.float32, tag="a4")
                    z4 = pool.tile([128, 3, H], dt.float32, tag="z4")
                    if has_bias:
                        nc.vector.tensor_tensor(
                            out=z4[:, 0:k3, :], in0=psm[:, 0:k3, F:F2],
                            in1=brow_bc[:, F:F2][:, None, :]
                                .broadcast_to((128, k3, H)),
                            op=AOP.add)
                    else:
                        nc.scalar.activation(z4[:, 0:k3, :], psm[:, 0:k3, F:F2],
                                             mybir.ActivationFunctionType.Copy)
                    nc.vector.scalar_tensor_tensor(
                        out=a4[:, 0:k3, :], in0=z4[:, 0:k3, :], scalar=0.01,
                        in1=z4[:, 0:k3, :], op0=AOP.mult, op1=AOP.max)
                    nc.scalar.activation(msgu[:, 0:k3, F:F2], a4[:, 0:k3, :],
                                         mybir.ActivationFunctionType.Exp,
                                         bias=nshift[:, :])
                    if has_bias:
                        pb = pool.tile([128, 3, F], dt.float32, tag="pb")
                        nc.vector.tensor_tensor(
                            out=pb[:, 0:k3, :], in0=psm[:, 0:k3, 0:F],
                            in1=brow_bc[:, 0:F][:, None, :]
                                .broadcast_to((128, k3, F)),
                            op=AOP.add)
                        src_part = pb[:, 0:k3, 0:F]
                    else:
                        src_part = psm[:, 0:k3, 0:F]
                    nc.vector.tensor_tensor(
                        out=msgu[:, 0:k3, 0:F].rearrange(
                            "p k (h d) -> p k h d", h=H),
                        in0=src_part.rearrange("p k (h d) -> p k h d", h=H),
                        in1=msgu[:, 0:k3, F:F2][:, :, :, None]
                            .broadcast_to((128, k3, H, DH)),
                        op=AOP.mult)
                    for k in range(k3):
                        t = t3 * 3 + k
                        nc.tensor.matmul(psb, lhsT=Pb[:, t, :],
                                         rhs=msgu[:, k, :],
                                         start=(t == 0), stop=(t == tpb - 1),
                                         skip_group_check=True)

                # ---------- phase 3 for block g ----------
                ss = pool.tile([128, H], dt.float32, tag="ss")
                nc.vector.tensor_scalar(out=ss, in0=psb[:, F:F2],
                                        scalar1=1e-30, scalar2=None,
                                        op0=AOP.max)
                inv = pool.tile([128, H], dt.float32, tag="inv")
                nc.vector.reciprocal(inv, ss)
                red = pool.tile([128, 1], dt.float32, tag="red")
                nc.vector.reduce_sum(red, psb[:, F:F2],
                                     axis=mybir.AxisListType.X)
                msk = pool.tile([128, 1], dt.float32, tag="msk")
                nc.vector.tensor_scalar(out=msk, in0=red, scalar1=0.0,
                                        scalar2=None, op0=AOP.is_gt)
                mn = pool.tile([128, F], dt.float32, tag="mn")
                nc.vector.tensor_tensor(
                    out=mn[:, :].rearrange("p (h d) -> p h d", h=H),
                    in0=psb[:, 0:F].rearrange("p (h d) -> p h d", h=H),
                    in1=inv[:, :, None].broadcast_to((128, H, DH)),
                    op=AOP.mult)
                # s1 = y3*msk + mn, s2 = nftres + s1
                s1 = pool.tile([128, F], dt.float32, tag="s1")
                nc.vector.scalar_tensor_tensor(
                    out=s1, in0=y3tab[:, g, :], scalar=msk[:, :], in1=mn,
                    op0=AOP.mult, op1=AOP.add)
                s2 = pool.tile([128, F], dt.float32, tag="s2")
                nc.vector.tensor_tensor(out=s2, in0=s1, in1=nftres_s[:, g, :],
                                        op=AOP.add)
                oc = pool.tile([128, F], dt.float32, tag="oc")
                nc.scalar.activation(oc, s2, mybir.ActivationFunctionType.Relu)
                nc.sync.dma_start(out=out_d[g * 128:(g + 1) * 128, :], in_=oc)

    nc.compile()
    return nc


def pack_nodes(dst, n_nodes, edge_cap):
    """LPT bin-packing of nodes into NBLK blocks (<=NODE_CAP nodes,
    <=edge_cap in-edges). Returns (block_of_node, slot_of_node, nodes_per
    [NBLK, NODE_CAP] with -1 padding) or None if infeasible."""
    import heapq
    deg = np.bincount(dst, minlength=n_nodes).astype(np.int64)
    order = np.argsort(-deg, kind="stable")
    heap = [(0, b) for b in range(NBLK)]
    heapq.heapify(heap)
    counts = np.zeros(NBLK, dtype=np.int64)
    loads = np.zeros(NBLK, dtype=np.int64)
    block_of = np.empty(n_nodes, dtype=np.int32)
    slot_of = np.empty(n_nodes, dtype=np.int32)
    spill = []
    for n in order:
        d = int(deg[n])
        placed = False
        while heap:
            load, b = heapq.heappop(heap)
            if counts[b] >= NODE_CAP:
                continue  # node-full: drop from heap permanently
            if load + d <= edge_cap:
                block_of[n] = b
                slot_of[n] = counts[b]
                counts[b] += 1
                loads[b] = load + d
                heapq.heappush(heap, (loads[b], b))
                placed = True
                break
            else:
                spill.append((load, b))
        for item in spill:
            heapq.heappush(heap, item)
        spill.clear()
        if not placed:
            return None
    nodes_per = np.full((NBLK, NODE_CAP), -1, dtype=np.int64)
    nodes_per[block_of, slot_of] = np.arange(n_nodes)
    return block_of, slot_of, nodes_per


def prep_inputs(nft, eft, W_path, b_path, W_attn1, attn2, src, dst, tpb,
                block_of, slot_of, nodes_per):
    n_nodes = nft.shape[0]
    edge_cap = tpb * 128
    epc = BPC * edge_cap
    E = eft.shape[0]

    nft16 = nft.astype(np.float16)
    eft16 = eft.astype(np.float16)

    # global edge slotting: position = block*edge_cap + rank_within_block
    eb = block_of[dst]
    eorder = np.argsort(eb, kind="stable")
    counts = np.bincount(eb, minlength=NBLK)
    assert counts.max() <= edge_cap
    starts = np.zeros(NBLK + 1, dtype=np.int64)
    np.cumsum(counts, out=starts[1:])
    rank = np.arange(E, dtype=np.int64) - starts[eb[eorder]]
    pos = eb[eorder] * edge_cap + rank          # slot for edge eorder[i]

    TOT = NBLK * edge_cap
    eft_all = np.zeros((TOT, F), dtype=np.float16)
    nfts_all = np.zeros((TOT, F), dtype=np.float16)
    nftd_all = np.zeros((TOT, F), dtype=np.float16)
    dstloc_all = np.full(TOT, DUMMY_SLOT, dtype=np.float16)
    es = eorder
    eft_all[pos] = eft16[es]
    nfts_all[pos] = nft16[src[es]]
    nftd_all[pos] = nft16[dst[es]]
    dstloc_all[pos] = slot_of[dst[es]].astype(np.float16)

    # weights (host-side pure weight algebra, like the baseline's A2blk)
    a2 = np.asarray(attn2, dtype=np.float32).reshape(H, DH)
    A2blk = np.zeros((F, H), dtype=np.float32)
    for h in range(H):
        A2blk[h * DH:(h + 1) * DH, h] = a2[h]
    Wp = np.asarray(W_path, dtype=np.float32)
    W1, W2, W3 = Wp[0:F], Wp[F:2 * F], Wp[2 * F:3 * F]
    W1qa = np.concatenate([W1, W1 @ A2blk + np.asarray(W_attn1, np.float32)],
                          axis=1).astype(np.float16)
    W2cat = np.concatenate([W2, W2 @ A2blk], axis=1).astype(np.float16)
    W3A2 = (W3 @ A2blk).astype(np.float16)
    W3_16 = W3.astype(np.float16)

    b = np.asarray(b_path, dtype=np.float32).reshape(F)
    has_bias = bool(np.any(b != 0))
    brow = np.concatenate([b, b @ A2blk]).reshape(1, F2).astype(np.float16)

    in_maps = []
    for c in range(NCORES):
        lo = c * BPC * edge_cap
        hi = lo + epc
        own = nodes_per[c * BPC:(c + 1) * BPC].reshape(-1)  # [NPC] node ids
        ok = own >= 0
        nftres = np.zeros((NPC, F), dtype=np.float16)
        nftres[ok] = nft16[own[ok]]
        m = {
            "eftT": np.ascontiguousarray(eft_all[lo:hi].T),
            "nftsT": np.ascontiguousarray(nfts_all[lo:hi].T),
            "nftdT": np.ascontiguousarray(nftd_all[lo:hi].T),
            "dstT": np.ascontiguousarray(
                dstloc_all[lo:hi].reshape(-1, 128).T),
            "nftT_own": np.ascontiguousarray(nftres.T),
            "nftres": nftres,
            "W1qa": W1qa,
            "W2cat": W2cat,
            "W3A2": W3A2,
            "W3": W3_16,
        }
        if has_bias:
            m["brow"] = brow
        in_maps.append(m)
    return in_maps, has_bias


_NC_CACHE = {}


def _get_nc(key, *args):
    if key not in _NC_CACHE:
        _NC_CACHE[key] = build_nc(*args)
    return _NC_CACHE[key]


def run(nft, eft, W_path, b_path, W_attn1, attn2, src, dst, trace=False,
        tmpdir=None, prec=None):
    nft = np.asarray(nft, dtype=np.float32)
    eft = np.asarray(eft, dtype=np.float32)
    src = np.asarray(src, dtype=np.int64)
    dst = np.asarray(dst, dtype=np.int64)
    n_nodes = nft.shape[0]
    assert n_nodes <= NBLK * NODE_CAP

    tpb = 16
    packed = pack_nodes(dst, n_nodes, tpb * 128)
    while packed is None:
        tpb += 1
        packed = pack_nodes(dst, n_nodes, tpb * 128)
    block_of, slot_of, nodes_per = packed

    in_maps, has_bias = prep_inputs(nft, eft, np.asarray(W_path),
                                    np.asarray(b_path), np.asarray(W_attn1),
                                    np.asarray(attn2), src, dst, tpb,
                                    block_of, slot_of, nodes_per)
    nc = _get_nc((tpb, has_bias), tpb, has_bias)
    kw = {}
    if trace:
        kw = dict(trace=True, tmpdir=tmpdir)
    res = bass_utils.run_bass_kernel_spmd(nc, in_maps,
                                          core_ids=list(range(NCORES)), **kw)

    out = np.empty((n_nodes, F), dtype=np.float32)
    for c in range(NCORES):
        own = nodes_per[c * BPC:(c + 1) * BPC].reshape(-1)
        ok = own >= 0
        out[own[ok]] = res.results[c]["out"][ok]
    return out, res


def kernel(**inputs):
    out, _ = run(**inputs)
    return out
